# revision 27
# baseline (speedup 1.0000x reference)
"""MoE feed-forward (8 experts, hard argmin routing) on 8 TRN2 NeuronCores.

Strategy
--------
Host (numpy): rms_norm + argmin routing (0.13% of FLOPs), then a dispatch
plan: an exact-DP search picks a per-core SEGMENT-SIZE TEMPLATE (uniform
across cores, so one SPMD program serves all 8) sized to the actual
routing distribution -- per-core token slots ~1056 instead of the naive
128-tile padding's 1152.  Each (core, segment) slot is filled with one
expert's tokens (zero-padded tail); weights/activations are cast to bf16
on host (fp32 PSUM accumulation).

Device (Bass/Tile, SPMD x8): per segment, STREAM the expert weights
through [128, 4, 1024] SBUF chunks consumed by weight-stationary
matmuls.  up-proj (K=1024) -> swiglu (ACT Silu + DVE mul) -> down-proj
(K=2048), yT written back to DRAM in bf16.  A calibrated warm-up of
dependency-free matmuls ramps the PE clock while the first weight
chunks land; DMA descriptors are ordered so the first up-proj chunk
is issued first (Sync-engine descriptor issue is ~0.65us each).

Host: scatter y back to token order and add the skip connection.
"""

import json
import math
import random
from functools import lru_cache
from itertools import combinations_with_replacement

import ml_dtypes
import numpy as np

N_EXPERTS = 8
DIM = 1024
HID = 2048
N_CORES = 8
P = 128
EPS = 1e-6
N_WARM = 42  # 128-col dependency-free matmuls to ramp the PE clock
GPAD = 512  # SBUF tiles the PE streams from are padded to this inner dim
            # (power-of-2 row strides; odd strides halve the PE stream rate)

BF16 = ml_dtypes.bfloat16
F8E3 = ml_dtypes.float8_e3m4  # TRN FP8_EXP3: 4 mantissa bits, max +-15.5
E3MAX = 15.5
E3MARGIN = 0.97
W_E3M4 = False  # False: bf16 weights (exact); True: e3m4 (halved weight DMA)


# ----------------------------------------------------------------------------
# BIR fixup: walrus in this container accepts at most ONE sync-wait per
# instruction.  Split instructions with k>1 waits into (k-1) pure-wait
# EventSemaphore instructions on the same engine immediately before.
# ----------------------------------------------------------------------------
def _split_multiwait_json(bir_bytes: bytes) -> bytes:
    m = json.loads(bir_bytes)
    ctr = 0
    for func in m["functions"]:
        for bb in func["blocks"]:
            out = []
            for inst in bb["instructions"]:
                si = inst.get("sync_info")
                waits = (si or {}).get("on_wait") or []
                if len(waits) > 1:
                    for w in waits[:-1]:
                        ctr += 1
                        out.append({
                            "debug": inst.get("debug", 0),
                            "engine": inst["engine"],
                            "ins": [],
                            "outs": [],
                            "name": f"waitfix_{ctr}",
                            "opcode": "EventSemaphore",
                            "sync_info": {"on_update": [], "on_wait": [w]},
                        })
                    si["on_wait"] = [waits[-1]]
                out.append(inst)
            bb["instructions"] = out
    return json.dumps(m).encode()


def _patch_bass_json(nc):
    orig = nc.to_json_bytes

    def patched():
        return _split_multiwait_json(orig())

    nc.to_json_bytes = patched


# ----------------------------------------------------------------------------
# Host-side routing (replicates the reference numerics in fp32)
# ----------------------------------------------------------------------------
def _route(x, scale, centroids):
    xf = x.reshape(-1, DIM).astype(np.float32)
    ms = np.mean(xf * xf, axis=-1, keepdims=True)
    s = scale.astype(np.float32) / np.sqrt(ms + EPS)
    xn = xf * s
    nx = np.sum(xn * xn, axis=-1)[:, None]
    ny = np.sum(centroids * centroids, axis=-1)[None, :]
    d2 = nx + ny - 2.0 * (xn @ centroids.T)
    ids = np.argmin(d2, axis=-1).astype(np.int32)
    return xn, ids


# ----------------------------------------------------------------------------
# Dispatch planner: exact-DP segment-template search.
#
# One SPMD program runs on all cores, so segment sizes must be uniform
# across cores; which EXPERT fills each (core, segment) slot is data.
# Find the template (t_1..t_K) minimizing S = sum(t_k) such that the
# actual per-expert token counts can be packed into the 8xK slots
# (each slot holds tokens of at most one expert; slack is zero-padding).
# ----------------------------------------------------------------------------
def _solve_template(template, demands):
    """Exact feasibility via DFS+memo over remaining slot counts.
    Returns per-demand bundles (n_slots of each size) or None."""
    K = len(template)
    tmpl = tuple(template)

    def min_bundles(need, avail):
        out = []

        def rec(k, counts, cap):
            if cap >= need:
                out.append(tuple(counts + [0] * (K - len(counts))))
                return
            if k == K:
                return
            for n in range(avail[k] + 1):
                counts.append(n)
                rec(k + 1, counts, cap + n * tmpl[k])
                counts.pop()
                if cap + n * tmpl[k] >= need:
                    break
        rec(0, [], 0)
        res = []
        for b in set(out):
            cap = sum(n * t for n, t in zip(b, tmpl))
            if cap < need:
                continue
            if all(not (b[k] and cap - tmpl[k] >= need) for k in range(K)):
                res.append(b)
        res.sort(key=lambda b: sum(n * t for n, t in zip(b, tmpl)))
        return res

    @lru_cache(maxsize=None)
    def dfs(i, avail):
        if i == len(demands):
            return ()
        for b in min_bundles(demands[i], avail):
            if all(b[k] <= avail[k] for k in range(K)):
                rest = dfs(i + 1, tuple(a - n for a, n in zip(avail, b)))
                if rest is not None:
                    return (b,) + rest
        return None

    return dfs(0, (N_CORES,) * K)


def _plan(ids):
    tok_by_e = [np.where(ids == e)[0] for e in range(N_EXPERTS)]
    counts = sorted(((len(t), e) for e, t in enumerate(tok_by_e) if len(t)),
                    reverse=True)
    demands = tuple(c for c, _ in counts)

    # Heavy DMA caps the PE clock (~2.0GHz at 40MB/core vs ~2.39 at 27MB),
    # so bound per-core weight traffic: K segments x 6.3MB (e3m4 weights)
    # + ~2.4MB activations must stay comfortably under the known-good 27.5MB.
    def best_for_K(K, s_max):
        cands = []
        for combo in combinations_with_replacement(range(1024, 63, -32), K):
            S = sum(combo)
            if 1024 <= S <= s_max:
                cands.append((S, combo))
        cands.sort()
        for S, combo in cands:
            r = _solve_template(combo, demands)
            if r is not None:
                return S, combo, r
        return None

    template = bundles = None
    for K in (2, 3):
        got = best_for_K(K, 2048 if K == 2 else 1600)
        if got is not None:
            S, combo, r = got
            if template is None or S < sum(template):
                template, bundles = combo, r
    assert template is not None, "template search failed"
    # order segments so the final group (and the output tail) is smallest
    perm = sorted(range(len(template)),
                  key=lambda k: -_groups_of(template[k])[-1])
    template = tuple(template[k] for k in perm)
    bundles = tuple(tuple(b[k] for k in perm) for b in bundles)

    # materialize bundles -> (core, seg) -> (expert, n_tokens)
    K = len(template)
    free = {k: list(range(N_CORES)) for k in range(K)}
    assign = {}
    for (cnt, e), bundle in zip(counts, bundles):
        rem = cnt
        # use slots largest-size-first so the partial slot is the smallest
        for k in sorted(range(K), key=lambda k: -template[k]):
            for _ in range(bundle[k]):
                c = free[k].pop()
                take = min(rem, template[k])
                assign[(c, k)] = (e, take)
                rem -= take
        assert rem == 0
    return template, assign, tok_by_e


# ----------------------------------------------------------------------------
# Device program
# ----------------------------------------------------------------------------
def _groups_of(t):
    out = []
    while t > 512:
        out.append(512)
        t -= 512
    if t:
        out.append(t)
    return out


def _build_program(template, inv_sg, s_pad):
    import concourse.bass as bass
    import concourse.mybir as mybir
    import concourse.tile as tile

    f32 = mybir.dt.float32
    bf16 = mybir.dt.bfloat16
    f8e3 = mybir.dt.float8e3 if W_E3M4 else mybir.dt.bfloat16
    Silu = mybir.ActivationFunctionType.Silu

    K = len(template)
    S = s_pad

    nc = bass.Bass("TRN2", debug=False)
    xnt_in = nc.dram_tensor("xnt", [P, 8, S], bf16, kind="ExternalInput").ap()
    # up weights (e3m4): per (segment, quad q, kq of 2): [128, 4 koi, 1024]
    # where 1024 = cols [a(4q)..a(4q+3) | g(4q)..g(4q+3)].
    up_in = nc.dram_tensor("up", [K, 4, 2, P, 4, 1024], f8e3,
                           kind="ExternalInput").ap()
    # down weights (e3m4): per (segment, kq of 4): [128, 4 khi, 1024].
    down_in = nc.dram_tensor("down", [K, 4, P, 4, 1024], f8e3,
                             kind="ExternalInput").ap()
    yt_out = nc.dram_tensor("yt", [P, 8, S], bf16, kind="ExternalOutput").ap()

    with tile.TileContext(nc) as tc:
        with (
            tc.tile_pool(name="upw", bufs=10) as up_pool,
            tc.tile_pool(name="dnw", bufs=6) as dn_pool,
            tc.tile_pool(name="xnf", bufs=6) as xn_pool,
            tc.tile_pool(name="act", bufs=2) as act_pool,
            tc.tile_pool(name="yc", bufs=1) as yc_pool,
            tc.tile_pool(name="warm", bufs=1) as warm_pool,
            tc.tile_pool(name="ps", bufs=8, space="PSUM") as ps,
        ):
            # ---- PE warm-up: ramp the clock while first DMAs land ----
            wsrc = warm_pool.tile([P, 256], bf16, tag="warm")
            nc.vector.memset(wsrc[:], 0.0)
            wps = [ps.tile([P, GPAD], f32, tag="ps", name=f"wps{i}")
                   for i in range(2)]
            for i in range(N_WARM):
                nc.tensor.matmul(wps[i % 2][:, 0:P], wsrc[:, 0:P],
                                 wsrc[:, P : 2 * P], start=True, stop=True)

            # ---- DMA issue schedule ----
            upt = {}
            dnt = {}
            xn_t = {}

            def dma_up(s, q, kq):
                w = up_pool.tile([P, 4, 1024], f8e3, tag="upw",
                                 name=f"up_{s}_{q}_{kq}")
                nc.sync.dma_start(w[:], up_in[s, q, kq])
                upt[(s, q, kq)] = w

            def dma_dn(s, kq):
                w = dn_pool.tile([P, 4, 1024], f8e3, tag="dnw",
                                 name=f"dn_{s}_{kq}")
                nc.sync.dma_start(w[:], down_in[s, kq])
                dnt[(s, kq)] = w

            def dma_xn(s, gi, half, col0, gn):
                # per-group xn, padded to GPAD cols (pow2 row stride)
                t = xn_pool.tile([P, 4, GPAD], bf16, tag="xnf",
                                 name=f"xn{s}_{gi}_{half}")
                nc.sync.dma_start(
                    t[:, :, 0:gn],
                    xnt_in[:, 4 * half : 4 * half + 4, col0 : col0 + gn])
                xn_t[(s, gi, half)] = t

            def seg_groups(s):
                col0 = sum(template[:s])
                out = []
                for gi, gn in enumerate(_groups_of(template[s])):
                    out.append((gi, col0, gn))
                    col0 += gn
                return out

            # critical order: first up chunk first, xn(seg0 group0) next
            dma_up(0, 0, 0)
            for gi, col0, gn in seg_groups(0):
                dma_xn(0, gi, 0, col0, gn)
                dma_xn(0, gi, 1, col0, gn)
            dma_up(0, 0, 1)
            for q in (1, 2, 3):
                dma_up(0, q, 0)
                dma_up(0, q, 1)
            for kq in range(4):
                dma_dn(0, kq)
            for s in range(1, K):
                for gi, col0, gn in seg_groups(s):
                    dma_xn(s, gi, 0, col0, gn)
                    dma_xn(s, gi, 1, col0, gn)
                for q in range(4):
                    dma_up(s, q, 0)
                    dma_up(s, q, 1)
                for kq in range(4):
                    dma_dn(s, kq)

            # ---- compute ----
            col = 0
            for s in range(K):
                for gi, gn in enumerate(_groups_of(template[s])):
                    is_last = (s == K - 1
                               and gi == len(_groups_of(template[s])) - 1)
                    act_t = act_pool.tile([P, 16, GPAD], bf16, tag="act")
                    # up projection: 4 quads x 4 subs x (8 ko x 2 mm)
                    for q in range(4):
                        for sub in range(4):
                            pa0 = ps.tile([P, GPAD], f32, tag="ps")
                            pg0 = ps.tile([P, GPAD], f32, tag="ps")
                            ca = sub * P
                            cg = 512 + sub * P
                            for ko in range(8):
                                w = upt[(s, q, ko // 4)][:, ko % 4, :]
                                xr = xn_t[(s, gi, ko // 4)][:, ko % 4, 0:gn]
                                first, last = ko == 0, ko == 7
                                nc.tensor.matmul(pa0[:, 0:gn],
                                                 w[:, ca : ca + P],
                                                 xr, start=first, stop=last)
                                nc.tensor.matmul(pg0[:, 0:gn],
                                                 w[:, cg : cg + P],
                                                 xr, start=first, stop=last)
                            j = 4 * q + sub
                            nc.scalar.activation(act_t[:, j, 0:gn],
                                                 pg0[:, 0:gn], Silu,
                                                 scale=inv_sg)
                            nc.vector.tensor_mul(act_t[:, j, 0:gn],
                                                 pa0[:, 0:gn],
                                                 act_t[:, j, 0:gn])
                    # down projection: 4 rounds x (16 kh x 2 matmuls)
                    yc = yc_pool.tile([P, 8, GPAD], bf16,
                                      tag="ycl" if is_last else "yc",
                                      name="yc")
                    for rr in range(4):
                        pd = [ps.tile([P, GPAD], f32, tag="ps",
                                      name=f"pd{q}")
                              for q in range(2)]
                        for kh in range(16):
                            w = dnt[(s, kh // 4)][:, kh % 4, :]
                            first, last = kh == 0, kh == 15
                            for q in range(2):
                                c = (2 * rr + q) * P
                                nc.tensor.matmul(pd[q][:, 0:gn],
                                                 w[:, c : c + P],
                                                 act_t[:, kh, 0:gn],
                                                 start=first, stop=last)
                        for q in range(2):
                            nc.vector.tensor_copy(yc[:, 2 * rr + q, 0:gn],
                                                  pd[q][:, 0:gn])
                        if is_last:
                            # finest-grained output on the final group so
                            # the tail DMA starts as early as possible
                            nc.sync.dma_start(
                                yt_out[:, 2 * rr : 2 * rr + 2,
                                       col : col + gn],
                                yc[:, 2 * rr : 2 * rr + 2, 0:gn])
                        elif rr % 2 == 1:
                            h = 2 * rr - 2
                            nc.sync.dma_start(
                                yt_out[:, h : h + 4, col : col + gn],
                                yc[:, h : h + 4, 0:gn])
                    col += gn

    _patch_bass_json(nc)
    return nc


# ----------------------------------------------------------------------------
# Host-side weight packing into the streaming layouts
# ----------------------------------------------------------------------------
def _pack_up(up_e):
    """[DIM, 2H] -> [4 q, 2 kq, 128, 4 koi, 1024]."""
    U = up_e.reshape(8, P, 2 * HID)
    A = U[:, :, :HID].reshape(8, P, 16, P)
    G = U[:, :, HID:].reshape(8, P, 16, P)
    out = np.empty((4, 8, P, 1024), dtype=up_e.dtype)
    for q in range(4):
        for i in range(4):
            out[q, :, :, i * P : (i + 1) * P] = A[:, :, 4 * q + i]
            out[q, :, :, 512 + i * P : 512 + (i + 1) * P] = G[:, :, 4 * q + i]
    return np.ascontiguousarray(
        out.reshape(4, 2, 4, P, 1024).transpose(0, 1, 3, 2, 4)
    )


def _pack_down(down_e):
    """[HID, DIM] -> [4 kq, 128, 4 khi, 1024]."""
    D = down_e.reshape(4, 4, P, DIM)
    return np.ascontiguousarray(D.transpose(0, 2, 1, 3))


# ----------------------------------------------------------------------------
# Entry point
# ----------------------------------------------------------------------------
def _run(inputs, trace=False, tmpdir=None):
    from concourse.bass_utils import run_bass_kernel_spmd

    x = np.asarray(inputs["x"])
    scale = np.asarray(inputs["scale"])
    centroids = np.asarray(inputs["centroids"])
    up_w = np.asarray(inputs["up_w"])
    down_w = np.asarray(inputs["down_w"])

    B, Sq, D = x.shape
    ntok = B * Sq
    xf32 = x.reshape(ntok, D).astype(np.float32)

    xn, ids = _route(x, scale, centroids)
    template, assign, tok_by_e = _plan(ids)
    K = len(template)
    S = sum(template)
    col_of = np.cumsum([0] + list(template))

    # e3m4 weight quantization with GLOBAL (over experts) scales, so the
    # silu dequant is one compile-time constant and the output dequant is
    # one host-side multiply.
    if W_E3M4:
        upf = up_w.astype(np.float32)
        dwf = down_w.astype(np.float32)
        s_a = E3MAX * E3MARGIN / np.abs(upf[:, :, :HID]).max()
        s_g = E3MAX * E3MARGIN / np.abs(upf[:, :, HID:]).max()
        s_d = E3MAX * E3MARGIN / np.abs(dwf).max()
        up_q = np.empty_like(upf)
        up_q[:, :, :HID] = upf[:, :, :HID] * s_a
        up_q[:, :, HID:] = upf[:, :, HID:] * s_g
        up_q = np.clip(up_q, -E3MAX, E3MAX).astype(F8E3)
        dw_q = np.clip(dwf * s_d, -E3MAX, E3MAX).astype(F8E3)
    else:
        s_a = s_g = s_d = 1.0
        up_q = up_w.astype(BF16)
        dw_q = down_w.astype(BF16)

    up_packed_e = {}
    down_packed_e = {}
    for e in range(N_EXPERTS):
        if any(v[0] == e for v in assign.values()):
            up_packed_e[e] = _pack_up(up_q[e])
            down_packed_e[e] = _pack_down(dw_q[e])

    s_pad = 2048  # pow2 DRAM row stride for xnt/yt
    xnT = np.ascontiguousarray(xn.T)  # [DIM, ntok] f32
    cursor = [0] * N_EXPERTS
    core_cols_tok = [np.zeros(S, dtype=np.int64) for _ in range(N_CORES)]
    core_cols_valid = [np.zeros(S, dtype=bool) for _ in range(N_CORES)]
    in_maps = []
    wdt = F8E3 if W_E3M4 else BF16
    for c in range(N_CORES):
        up_pack = np.zeros((K, 4, 2, P, 4, 1024), dtype=wdt)
        down_pack = np.zeros((K, 4, P, 4, 1024), dtype=wdt)
        for k in range(K):
            if (c, k) not in assign:
                continue
            e, take = assign[(c, k)]
            up_pack[k] = up_packed_e[e]
            down_pack[k] = down_packed_e[e]
            toks = tok_by_e[e]
            sel = toks[cursor[e] : cursor[e] + take]
            cursor[e] += take
            col = col_of[k]
            core_cols_tok[c][col : col + take] = sel
            core_cols_valid[c][col : col + take] = True
        xnt_cols = np.where(core_cols_valid[c][None, :],
                            xnT[:, core_cols_tok[c]], 0.0).astype(BF16)
        xnt_pack = np.zeros((P, 8, s_pad), dtype=BF16)
        xnt_pack[:, :, :S] = xnt_cols.reshape(8, P, S).transpose(1, 0, 2)
        in_maps.append({"xnt": xnt_pack, "up": up_pack, "down": down_pack})

    for e in range(N_EXPERTS):
        assert cursor[e] == len(tok_by_e[e]), "dispatch did not cover all tokens"

    nc = _build_program(template, float(1.0 / s_g), s_pad)
    kwargs = {}
    if trace:
        kwargs = dict(trace=True, tmpdir=tmpdir)
    res = run_bass_kernel_spmd(nc, in_maps, core_ids=list(range(N_CORES)),
                               **kwargs)

    # ---- scatter + skip (undo the e3m4 weight scales) ----
    deq = np.float32(1.0 / (s_a * s_d))
    out = xf32.copy()
    for c in range(N_CORES):
        yt = np.ascontiguousarray(
            res.results[c]["yt"][:, :, :S].astype(np.float32)
            .reshape(P, 8, S).transpose(1, 0, 2)
        ).reshape(8 * P, S) * deq  # [DIM, S]
        valid = core_cols_valid[c]
        toks = core_cols_tok[c][valid]
        out[toks] = xf32[toks] + yt[:, valid].T
    return out.reshape(B, Sq, D).astype(x.dtype), res


def kernel(**inputs) -> np.ndarray:
    out, _ = _run(inputs)
    return out
